# revision 18
# baseline (speedup 1.0000x reference)
"""Trainium2 Bass kernel for nn_DensityGrid.

Reference computation on a [96,96,96] float32 grid:
  out_density = 1 - exp(-0.01 * relu(density))
  new_cached  = max(0.8 * density_cached, relu(density))
  field       = maxpool3d(1 - exp(-0.01 * new_cached), k=3, s=1, p=1)
  mask        = field > min(mean(field), 0.01)
  new_field   = largest connected component of mask (26-connectivity; the
                reference runs a 288-iteration masked max-dilation)
  valid       = new_field if step < 500 else old_field

Sharding: z-axis split across 8 NeuronCores, 12 planes per core, processed
as two 6-plane chunks so DMA / ScalarE / VectorE overlap. Host passes shards
pre-permuted to [y,z,x] so every DMA is a contiguous-row transfer.

Device-side algebra (per core):
  * m = max(0.8*c, d) via one fused scalar_tensor_tensor; new_cached is then
    just max(m, 0) and out_density = relu(1 - exp(-0.01*d)) (one Exp + one
    fused affine-Relu activation) == 1 - exp(-0.01*relu(d)) exactly.
  * CCL short-circuit: mask = field > min(mean(field), 0.01) and
    min(mean,0.01) <= 0.01, so `field > 0.01 everywhere` makes the mask
    all-True regardless of the mean; the reference's masked max-dilation then
    provably converges to the constant G^3 label inside its 288 iterations
    (grid L-inf diameter is 95), i.e. new_field is exactly all-True.
  * The all-True proof is computed in m-domain, f32-exact, with one
    sliding pairwise max plus a min-reduction per chunk:
        stat = min over shard of max(m[..., x], m[..., x+1])
    Every voxel's 3x3x3 pool window contains such an x-pair, so
    maxpool3d(m') >= pairmax everywhere (m' = relu(m) = new_cached, and the
    pair values are positive whenever the check passes). Host condition
    stat > 1.006 > -100*ln(0.99) then guarantees
    field = 1 - exp(-0.01*maxpool(new_cached)) > 0.01 everywhere even after
    the reference's f32 exp rounding. If the check fails, an exact NumPy
    replication of the reference computes new_field (not taken for this
    workload's data distribution: actual stat ~ 3.5).
"""

import sys

for _p in ("/opt/trn_rl_repo", "/root/.axon_site/_ro/trn_rl_repo"):
    if _p not in sys.path:
        sys.path.append(_p)

import numpy as np

G = 96
NCORES = 8
ZS = G // NCORES          # 12 planes per core
MTHR = 1.006              # m-domain acceptance threshold (-100*ln(0.99)=1.00503)

_CACHE = {}


def _build_program():
    import concourse.bass as bass
    from concourse import bacc, mybir
    import concourse.tile as tile

    f32 = mybir.dt.float32
    Alu = mybir.AluOpType
    Act = mybir.ActivationFunctionType

    nc = bacc.Bacc("TRN2", target_bir_lowering=False, debug=False,
                   num_devices=NCORES)

    # Host supplies/consumes [y,z,x] layout so every DMA is contiguous.
    d_in = nc.declare_dram_parameter("d", [G, ZS, G], f32, isOutput=False)
    c_in = nc.declare_dram_parameter("c", [G, ZS, G], f32, isOutput=False)
    outd = nc.declare_dram_parameter("outd", [G, ZS, G], f32, isOutput=True)
    outc = nc.declare_dram_parameter("outc", [G, ZS, G], f32, isOutput=True)
    stats = nc.declare_dram_parameter("stats", [G, 2], f32, isOutput=True)

    d_ap = d_in.ap()
    c_ap = c_in.ap()
    outd_ap = outd.ap()
    outc_ap = outc.ap()

    with tile.TileContext(nc) as tc:
        with (
            tc.tile_pool(name="io", bufs=1) as io,
            tc.tile_pool(name="work", bufs=1) as work,
        ):
            t_stats = work.tile([G, 2], f32, tag="stats")

            ZC = ZS // 2   # planes per chunk
            tiles = []
            # stat chain first (higher scheduler priority) ...
            for ch in range(2):
                zlo = ch * ZC
                t_d = io.tile([G, ZC, G], f32, tag=f"d{ch}")
                t_c = io.tile([G, ZC, G], f32, tag=f"c{ch}")
                nc.sync.dma_start(out=t_d[:], in_=d_ap[:, zlo:zlo + ZC, :])
                nc.sync.dma_start(out=t_c[:], in_=c_ap[:, zlo:zlo + ZC, :])

                # m = max(0.8*c, d)
                t_m = work.tile([G, ZC, G], f32, tag=f"m{ch}")
                nc.vector.scalar_tensor_tensor(
                    t_m[:], t_c[:], 0.8, t_d[:], Alu.mult, Alu.max)
                # stat: min over the shard of sliding-pair maxes of m.
                # Every voxel's 3x3x3 pool window contains the x-pair
                # (x, x+1) (or (x-1, x) at the edge), so
                # min(pairmax) > T proves maxpool3d(m-field) clears T
                # everywhere; f32-exact, no exp needed on the stat path.
                t_r1 = work.tile([G, ZC, G - 1], f32, tag=f"r1{ch}")
                nc.vector.tensor_tensor(
                    t_r1[:], t_m[:, :, 0:G - 1], t_m[:, :, 1:G],
                    op=Alu.max)
                nc.vector.tensor_reduce(
                    t_stats[:, ch:ch + 1], t_r1[:],
                    axis=mybir.AxisListType.XY, op=Alu.min)
                tiles.append((zlo, t_d, t_c, t_m))
            nc.sync.dma_start(out=stats.ap(), in_=t_stats[:])

            # ... exact output paths second (fill engine slack)
            for ch in range(2):
                zlo, t_d, t_c, t_m = tiles[ch]
                t_nc = work.tile([G, ZC, G], f32, tag=f"nc{ch}")
                nc.vector.tensor_scalar_max(t_nc[:], t_m[:], 0.0)
                nc.sync.dma_start(out=outc_ap[:, zlo:zlo + ZC, :],
                                  in_=t_nc[:])
                # out_density = relu(1 - exp(-0.01*d))
                t_ed = work.tile([G, ZC, G], f32, tag=f"ed{ch}")
                nc.scalar.activation(t_ed[:], t_d[:], Act.Exp, scale=-0.01)
                t_od = work.tile([G, ZC, G], f32, tag=f"od{ch}")
                nc.scalar.activation(t_od[:], t_ed[:], Act.Relu,
                                     bias=1.0, scale=-1.0)
                nc.sync.dma_start(out=outd_ap[:, zlo:zlo + ZC, :],
                                  in_=t_od[:])

    nc.compile()
    return nc


def _get_program():
    if "nc" not in _CACHE:
        _CACHE["nc"] = _build_program()
    return _CACHE["nc"]


def _pool1(x, ax):
    pad = [(0, 0)] * 3
    pad[ax] = (1, 1)
    xp = np.pad(x, pad)
    sl = lambda s: tuple(
        slice(s, s + G) if i == ax else slice(None) for i in range(3))
    return np.maximum(np.maximum(xp[sl(0)], xp[sl(1)]), xp[sl(2)])


def _pool3(x):
    return _pool1(_pool1(_pool1(x, 0), 1), 2)


def _numpy_new_field(density, density_cached):
    """Exact NumPy replication of the reference's mask + CCL path."""
    d = np.maximum(density.astype(np.float32), np.float32(0.0))
    ncache = np.maximum(density_cached.astype(np.float32) * np.float32(0.8), d)
    field = _pool3((np.float32(1.0) - np.exp(-np.float32(0.01) * ncache)
                    ).astype(np.float32))
    thr = min(field.mean(dtype=np.float32), np.float32(0.01))
    mask = field > thr
    m = mask.astype(np.float32)
    comp = np.arange(1, G ** 3 + 1, dtype=np.float32).reshape(G, G, G) * m
    for _ in range(3 * G):
        new = _pool3(comp) * m
        if np.array_equal(new, comp):
            break
        comp = new
    labels = comp.astype(np.int32)
    counts = np.zeros(G ** 3 + 1, np.float32)
    np.add.at(counts, labels.ravel(), m.ravel())
    counts[0] = -1.0
    label = np.int32(counts.argmax())
    return labels == label


def kernel(density, density_cached, old_field, step):
    from concourse.bass_utils import run_bass_kernel_spmd

    density = np.ascontiguousarray(np.asarray(density, dtype=np.float32))
    density_cached = np.ascontiguousarray(
        np.asarray(density_cached, dtype=np.float32))
    old_field = np.asarray(old_field).astype(bool)
    step_i = int(np.asarray(step))

    in_maps = [
        {"d": np.ascontiguousarray(
            density[k * ZS:(k + 1) * ZS].transpose(1, 0, 2)),
         "c": np.ascontiguousarray(
            density_cached[k * ZS:(k + 1) * ZS].transpose(1, 0, 2))}
        for k in range(NCORES)
    ]

    nc = _get_program()
    res = run_bass_kernel_spmd(nc, in_maps, core_ids=list(range(NCORES)))
    _CACHE["last_results"] = res

    out_density = np.concatenate(
        [res.results[k]["outd"].transpose(1, 0, 2) for k in range(NCORES)],
        axis=0)
    new_cached = np.concatenate(
        [res.results[k]["outc"].transpose(1, 0, 2) for k in range(NCORES)],
        axis=0)
    stat_min = float(
        min(res.results[k]["stats"].min() for k in range(NCORES)))

    if stat_min > MTHR:
        # every voxel has an in-window pair with m > MTHR > -100*ln(0.99),
        # so field > 0.01 >= min(mean, 0.01) everywhere -> mask all-True
        # -> the reference CCL converges to all-True exactly.
        new_field = np.ones((G, G, G), dtype=bool)
    else:
        new_field = _numpy_new_field(density, density_cached)

    valid = new_field if step_i < 500 else old_field
    return (out_density, valid, new_field, new_cached)


# revision 24
# speedup vs baseline: 1.0821x; 1.0821x over previous
"""Trainium2 Bass kernel for nn_DensityGrid.

Reference computation on a [96,96,96] float32 grid:
  out_density = 1 - exp(-0.01 * relu(density))
  new_cached  = max(0.8 * density_cached, relu(density))
  field       = maxpool3d(1 - exp(-0.01 * new_cached), k=3, s=1, p=1)
  mask        = field > min(mean(field), 0.01)
  new_field   = largest connected component of mask (26-connectivity; the
                reference runs a 288-iteration masked max-dilation)
  valid       = new_field if step < 500 else old_field

Sharding: z-axis split across 8 NeuronCores, 12 planes per core, processed
as two 6-plane chunks so DMA / ScalarE / VectorE overlap. Host passes shards
pre-permuted to [y,z,x] so every DMA is a contiguous-row transfer.

Device-side algebra (per core):
  * m = max(0.8*c, d) via one fused scalar_tensor_tensor; new_cached is then
    just max(m, 0) and out_density = relu(1 - exp(-0.01*d)) (one Exp + one
    fused affine-Relu activation) == 1 - exp(-0.01*relu(d)) exactly.
  * CCL short-circuit: mask = field > min(mean(field), 0.01) and
    min(mean,0.01) <= 0.01, so `field > 0.01 everywhere` makes the mask
    all-True regardless of the mean; the reference's masked max-dilation then
    provably converges to the constant G^3 label inside its 288 iterations
    (grid L-inf diameter is 95), i.e. new_field is exactly all-True.
  * The all-True proof is computed in m-domain, f32-exact, with one
    sliding pairwise max plus a min-reduction per chunk:
        stat = min over shard of max(m[..., x], m[..., x+1])
    Every voxel's 3x3x3 pool window contains such an x-pair, so
    maxpool3d(m') >= pairmax everywhere (m' = relu(m) = new_cached, and the
    pair values are positive whenever the check passes). Host condition
    stat > 1.006 > -100*ln(0.99) then guarantees
    field = 1 - exp(-0.01*maxpool(new_cached)) > 0.01 everywhere even after
    the reference's f32 exp rounding. If the check fails, an exact NumPy
    replication of the reference computes new_field (not taken for this
    workload's data distribution: actual stat ~ 3.5).
"""

import sys

for _p in ("/opt/trn_rl_repo", "/root/.axon_site/_ro/trn_rl_repo"):
    if _p not in sys.path:
        sys.path.append(_p)

import numpy as np

G = 96
NCORES = 8
ZS = G // NCORES          # 12 planes per core
MTHR = 1.006              # m-domain acceptance threshold (-100*ln(0.99)=1.00503)

_CACHE = {}


def _build_program():
    import concourse.bass as bass
    from concourse import bacc, mybir
    import concourse.tile as tile

    f32 = mybir.dt.float32
    Alu = mybir.AluOpType
    Act = mybir.ActivationFunctionType

    nc = bacc.Bacc("TRN2", target_bir_lowering=False, debug=False,
                   num_devices=NCORES)

    # Host supplies/consumes [y,z,x] layout so every DMA is contiguous.
    d_in = nc.declare_dram_parameter("d", [G, ZS, G], f32, isOutput=False)
    c_in = nc.declare_dram_parameter("c", [G, ZS, G], f32, isOutput=False)
    outd = nc.declare_dram_parameter("outd", [G, ZS, G], f32, isOutput=True)
    outc = nc.declare_dram_parameter("outc", [G, ZS, G], f32, isOutput=True)
    stats = nc.declare_dram_parameter("stats", [G, 2], f32, isOutput=True)

    d_ap = d_in.ap()
    c_ap = c_in.ap()
    outd_ap = outd.ap()
    outc_ap = outc.ap()

    with tile.TileContext(nc) as tc:
        with (
            tc.tile_pool(name="io", bufs=1) as io,
            tc.tile_pool(name="work", bufs=1) as work,
        ):
            t_stats = work.tile([G, 2], f32, tag="stats")

            ZC = ZS // 2   # planes per chunk
            tiles = []
            for ch in range(2):
                zlo = ch * ZC
                t_d = io.tile([G, ZC, G], f32, tag=f"d{ch}")
                t_c = io.tile([G, ZC, G], f32, tag=f"c{ch}")
                nc.sync.dma_start(out=t_d[:], in_=d_ap[:, zlo:zlo + ZC, :])
                nc.sync.dma_start(out=t_c[:], in_=c_ap[:, zlo:zlo + ZC, :])
                tiles.append((zlo, t_d, t_c))

            # DVE chain, ordered so work gated only by d (which lands one
            # transfer earlier than c) runs first: new_cached comes straight
            # out of one fused op per chunk, and the stat runs on new_cached
            # itself (maxpool3d(new_cached) >= any in-window pair of it).
            rds = []
            for ch in range(2):
                zlo, t_d, t_c = tiles[ch]
                t_rd = work.tile([G, ZC, G], f32, tag=f"rd{ch}")
                nc.vector.tensor_scalar_max(t_rd[:], t_d[:], 0.0)
                rds.append(t_rd)
            ncs = []
            for ch in range(2):
                zlo, t_d, t_c = tiles[ch]
                # new_cached = max(0.8*c, relu(d))
                t_nc = work.tile([G, ZC, G], f32, tag=f"nc{ch}")
                nc.vector.scalar_tensor_tensor(
                    t_nc[:], t_c[:], 0.8, rds[ch][:], Alu.mult, Alu.max)
                nc.sync.dma_start(out=outc_ap[:, zlo:zlo + ZC, :],
                                  in_=t_nc[:])
                # stat: min over the shard of disjoint-pair maxes of
                # new_cached; every voxel's 3x3x3 pool window contains its
                # own x-pair {2i, 2i+1}, so min(pairmax) > T proves
                # maxpool3d(new_cached) clears T everywhere. f32-exact.
                t_r1 = work.tile([G, ZC, G // 2], f32, tag=f"r1{ch}")
                nc.vector.tensor_tensor(
                    t_r1[:], t_nc[:, :, 0:G - 1:2], t_nc[:, :, 1:G:2],
                    op=Alu.max)
                nc.vector.tensor_reduce(
                    t_stats[:, ch:ch + 1], t_r1[:],
                    axis=mybir.AxisListType.XY, op=Alu.min)
                ncs.append(t_nc)
            nc.sync.dma_start(out=stats.ap(), in_=t_stats[:])

            # out_density = relu(1 - exp(-0.01*d)) on ScalarE, batched by
            # activation function
            eds = []
            for ch in range(2):
                zlo, t_d, t_c = tiles[ch]
                t_ed = work.tile([G, ZC, G], f32, tag=f"ed{ch}")
                nc.scalar.activation(t_ed[:], t_d[:], Act.Exp, scale=-0.01)
                eds.append(t_ed)
            for ch in range(2):
                zlo, t_d, t_c = tiles[ch]
                t_od = work.tile([G, ZC, G], f32, tag=f"od{ch}")
                nc.scalar.activation(t_od[:], eds[ch][:], Act.Relu,
                                     bias=1.0, scale=-1.0)
                # issue outd from ScalarE's HWDGE ring: output-DMA issue
                # otherwise serializes on the SP sequencer
                nc.scalar.dma_start(out=outd_ap[:, zlo:zlo + ZC, :],
                                    in_=t_od[:])

    nc.compile()
    return nc


def _get_program():
    if "nc" not in _CACHE:
        _CACHE["nc"] = _build_program()
    return _CACHE["nc"]


def _pool1(x, ax):
    pad = [(0, 0)] * 3
    pad[ax] = (1, 1)
    xp = np.pad(x, pad)
    sl = lambda s: tuple(
        slice(s, s + G) if i == ax else slice(None) for i in range(3))
    return np.maximum(np.maximum(xp[sl(0)], xp[sl(1)]), xp[sl(2)])


def _pool3(x):
    return _pool1(_pool1(_pool1(x, 0), 1), 2)


def _numpy_new_field(density, density_cached):
    """Exact NumPy replication of the reference's mask + CCL path."""
    d = np.maximum(density.astype(np.float32), np.float32(0.0))
    ncache = np.maximum(density_cached.astype(np.float32) * np.float32(0.8), d)
    field = _pool3((np.float32(1.0) - np.exp(-np.float32(0.01) * ncache)
                    ).astype(np.float32))
    thr = min(field.mean(dtype=np.float32), np.float32(0.01))
    mask = field > thr
    m = mask.astype(np.float32)
    comp = np.arange(1, G ** 3 + 1, dtype=np.float32).reshape(G, G, G) * m
    for _ in range(3 * G):
        new = _pool3(comp) * m
        if np.array_equal(new, comp):
            break
        comp = new
    labels = comp.astype(np.int32)
    counts = np.zeros(G ** 3 + 1, np.float32)
    np.add.at(counts, labels.ravel(), m.ravel())
    counts[0] = -1.0
    label = np.int32(counts.argmax())
    return labels == label


def kernel(density, density_cached, old_field, step):
    from concourse.bass_utils import run_bass_kernel_spmd

    density = np.ascontiguousarray(np.asarray(density, dtype=np.float32))
    density_cached = np.ascontiguousarray(
        np.asarray(density_cached, dtype=np.float32))
    old_field = np.asarray(old_field).astype(bool)
    step_i = int(np.asarray(step))

    in_maps = [
        {"d": np.ascontiguousarray(
            density[k * ZS:(k + 1) * ZS].transpose(1, 0, 2)),
         "c": np.ascontiguousarray(
            density_cached[k * ZS:(k + 1) * ZS].transpose(1, 0, 2))}
        for k in range(NCORES)
    ]

    nc = _get_program()
    res = run_bass_kernel_spmd(nc, in_maps, core_ids=list(range(NCORES)))
    _CACHE["last_results"] = res

    out_density = np.concatenate(
        [res.results[k]["outd"].transpose(1, 0, 2) for k in range(NCORES)],
        axis=0)
    new_cached = np.concatenate(
        [res.results[k]["outc"].transpose(1, 0, 2) for k in range(NCORES)],
        axis=0)
    stat_min = float(
        min(res.results[k]["stats"].min() for k in range(NCORES)))

    if stat_min > MTHR:
        # every voxel has an in-window pair with m > MTHR > -100*ln(0.99),
        # so field > 0.01 >= min(mean, 0.01) everywhere -> mask all-True
        # -> the reference CCL converges to all-True exactly.
        new_field = np.ones((G, G, G), dtype=bool)
    else:
        new_field = _numpy_new_field(density, density_cached)

    valid = new_field if step_i < 500 else old_field
    return (out_density, valid, new_field, new_cached)


# revision 27
# speedup vs baseline: 1.0894x; 1.0067x over previous
"""Trainium2 Bass kernel for nn_DensityGrid.

Reference computation on a [96,96,96] float32 grid:
  out_density = 1 - exp(-0.01 * relu(density))
  new_cached  = max(0.8 * density_cached, relu(density))
  field       = maxpool3d(1 - exp(-0.01 * new_cached), k=3, s=1, p=1)
  mask        = field > min(mean(field), 0.01)
  new_field   = largest connected component of mask (26-connectivity; the
                reference runs a 288-iteration masked max-dilation)
  valid       = new_field if step < 500 else old_field

Sharding: z-axis split across 8 NeuronCores, 12 planes per core, processed
as two 6-plane chunks so DMA / ScalarE / VectorE overlap. Host passes shards
pre-permuted to [y,z,x] so every DMA is a contiguous-row transfer.

Device-side algebra (per core):
  * m = max(0.8*c, d) via one fused scalar_tensor_tensor; new_cached is then
    just max(m, 0) and out_density = relu(1 - exp(-0.01*d)) (one Exp + one
    fused affine-Relu activation) == 1 - exp(-0.01*relu(d)) exactly.
  * CCL short-circuit: mask = field > min(mean(field), 0.01) and
    min(mean,0.01) <= 0.01, so `field > 0.01 everywhere` makes the mask
    all-True regardless of the mean; the reference's masked max-dilation then
    provably converges to the constant G^3 label inside its 288 iterations
    (grid L-inf diameter is 95), i.e. new_field is exactly all-True.
  * The all-True proof is computed in m-domain, f32-exact, with one
    sliding pairwise max plus a min-reduction per chunk:
        stat = min over shard of max(m[..., x], m[..., x+1])
    Every voxel's 3x3x3 pool window contains such an x-pair, so
    maxpool3d(m') >= pairmax everywhere (m' = relu(m) = new_cached, and the
    pair values are positive whenever the check passes). Host condition
    stat > 1.006 > -100*ln(0.99) then guarantees
    field = 1 - exp(-0.01*maxpool(new_cached)) > 0.01 everywhere even after
    the reference's f32 exp rounding. If the check fails, an exact NumPy
    replication of the reference computes new_field (not taken for this
    workload's data distribution: actual stat ~ 3.5).
"""

import sys

for _p in ("/opt/trn_rl_repo", "/root/.axon_site/_ro/trn_rl_repo"):
    if _p not in sys.path:
        sys.path.append(_p)

import numpy as np

G = 96
NCORES = 8
ZS = G // NCORES          # 12 planes per core
MTHR = 1.006              # m-domain acceptance threshold (-100*ln(0.99)=1.00503)

_CACHE = {}


def _build_program():
    import concourse.bass as bass
    from concourse import bacc, mybir
    import concourse.tile as tile

    f32 = mybir.dt.float32
    Alu = mybir.AluOpType
    Act = mybir.ActivationFunctionType

    nc = bacc.Bacc("TRN2", target_bir_lowering=False, debug=False,
                   num_devices=NCORES)

    # Host supplies/consumes [y,z,x] layout so every DMA is contiguous.
    d_in = nc.declare_dram_parameter("d", [G, ZS, G], f32, isOutput=False)
    c_in = nc.declare_dram_parameter("c", [G, ZS, G], f32, isOutput=False)
    outd = nc.declare_dram_parameter("outd", [G, ZS, G], f32, isOutput=True)
    outc = nc.declare_dram_parameter("outc", [G, ZS, G], f32, isOutput=True)
    stats = nc.declare_dram_parameter("stats", [G, 2], f32, isOutput=True)

    d_ap = d_in.ap()
    c_ap = c_in.ap()
    outd_ap = outd.ap()
    outc_ap = outc.ap()

    with tile.TileContext(nc) as tc:
        with (
            tc.tile_pool(name="io", bufs=1) as io,
            tc.tile_pool(name="work", bufs=1) as work,
        ):
            t_stats = work.tile([G, 2], f32, tag="stats")

            ZC = ZS // 2   # planes per chunk
            # both d shards land before the c shards: the d-gated work
            # (relu chain on DVE, exp chain on ScalarE) front-runs while
            # the c-gated scalar_tensor_tensor waits anyway
            tiles = []
            for ch in range(2):
                zlo = ch * ZC
                t_d = io.tile([G, ZC, G], f32, tag=f"d{ch}")
                nc.sync.dma_start(out=t_d[:], in_=d_ap[:, zlo:zlo + ZC, :])
                tiles.append([zlo, t_d, None])
            for ch in range(2):
                zlo = ch * ZC
                t_c = io.tile([G, ZC, G], f32, tag=f"c{ch}")
                nc.sync.dma_start(out=t_c[:], in_=c_ap[:, zlo:zlo + ZC, :])
                tiles[ch][2] = t_c

            # DVE chain, ordered so work gated only by d (which lands one
            # transfer earlier than c) runs first: new_cached comes straight
            # out of one fused op per chunk, and the stat runs on new_cached
            # itself (maxpool3d(new_cached) >= any in-window pair of it).
            rds = []
            for ch in range(2):
                zlo, t_d, t_c = tiles[ch]
                t_rd = work.tile([G, ZC, G], f32, tag=f"rd{ch}")
                nc.vector.tensor_scalar_max(t_rd[:], t_d[:], 0.0)
                rds.append(t_rd)
            ncs = []
            for ch in range(2):
                zlo, t_d, t_c = tiles[ch]
                # new_cached = max(0.8*c, relu(d))
                t_nc = work.tile([G, ZC, G], f32, tag=f"nc{ch}")
                nc.vector.scalar_tensor_tensor(
                    t_nc[:], t_c[:], 0.8, rds[ch][:], Alu.mult, Alu.max)
                nc.sync.dma_start(out=outc_ap[:, zlo:zlo + ZC, :],
                                  in_=t_nc[:])
                # stat: min over the shard of disjoint-pair maxes of
                # new_cached; every voxel's 3x3x3 pool window contains its
                # own x-pair {2i, 2i+1}, so min(pairmax) > T proves
                # maxpool3d(new_cached) clears T everywhere. f32-exact.
                t_r1 = work.tile([G, ZC, G // 2], f32, tag=f"r1{ch}")
                nc.vector.tensor_tensor(
                    t_r1[:], t_nc[:, :, 0:G - 1:2], t_nc[:, :, 1:G:2],
                    op=Alu.max)
                nc.vector.tensor_reduce(
                    t_stats[:, ch:ch + 1], t_r1[:],
                    axis=mybir.AxisListType.XY, op=Alu.min)
                ncs.append(t_nc)
            nc.sync.dma_start(out=stats.ap(), in_=t_stats[:])

            # out_density = relu(1 - exp(-0.01*d)) on ScalarE, batched by
            # activation function
            eds = []
            for ch in range(2):
                zlo, t_d, t_c = tiles[ch]
                t_ed = work.tile([G, ZC, G], f32, tag=f"ed{ch}")
                nc.scalar.activation(t_ed[:], t_d[:], Act.Exp, scale=-0.01)
                eds.append(t_ed)
            for ch in range(2):
                zlo, t_d, t_c = tiles[ch]
                t_od = work.tile([G, ZC, G], f32, tag=f"od{ch}")
                nc.scalar.activation(t_od[:], eds[ch][:], Act.Relu,
                                     bias=1.0, scale=-1.0)
                # issue outd from ScalarE's HWDGE ring: output-DMA issue
                # otherwise serializes on the SP sequencer
                nc.scalar.dma_start(out=outd_ap[:, zlo:zlo + ZC, :],
                                    in_=t_od[:])

    nc.compile()
    return nc


def _get_program():
    if "nc" not in _CACHE:
        _CACHE["nc"] = _build_program()
    return _CACHE["nc"]


def _pool1(x, ax):
    pad = [(0, 0)] * 3
    pad[ax] = (1, 1)
    xp = np.pad(x, pad)
    sl = lambda s: tuple(
        slice(s, s + G) if i == ax else slice(None) for i in range(3))
    return np.maximum(np.maximum(xp[sl(0)], xp[sl(1)]), xp[sl(2)])


def _pool3(x):
    return _pool1(_pool1(_pool1(x, 0), 1), 2)


def _numpy_new_field(density, density_cached):
    """Exact NumPy replication of the reference's mask + CCL path."""
    d = np.maximum(density.astype(np.float32), np.float32(0.0))
    ncache = np.maximum(density_cached.astype(np.float32) * np.float32(0.8), d)
    field = _pool3((np.float32(1.0) - np.exp(-np.float32(0.01) * ncache)
                    ).astype(np.float32))
    thr = min(field.mean(dtype=np.float32), np.float32(0.01))
    mask = field > thr
    m = mask.astype(np.float32)
    comp = np.arange(1, G ** 3 + 1, dtype=np.float32).reshape(G, G, G) * m
    for _ in range(3 * G):
        new = _pool3(comp) * m
        if np.array_equal(new, comp):
            break
        comp = new
    labels = comp.astype(np.int32)
    counts = np.zeros(G ** 3 + 1, np.float32)
    np.add.at(counts, labels.ravel(), m.ravel())
    counts[0] = -1.0
    label = np.int32(counts.argmax())
    return labels == label


def kernel(density, density_cached, old_field, step):
    from concourse.bass_utils import run_bass_kernel_spmd

    density = np.ascontiguousarray(np.asarray(density, dtype=np.float32))
    density_cached = np.ascontiguousarray(
        np.asarray(density_cached, dtype=np.float32))
    old_field = np.asarray(old_field).astype(bool)
    step_i = int(np.asarray(step))

    in_maps = [
        {"d": np.ascontiguousarray(
            density[k * ZS:(k + 1) * ZS].transpose(1, 0, 2)),
         "c": np.ascontiguousarray(
            density_cached[k * ZS:(k + 1) * ZS].transpose(1, 0, 2))}
        for k in range(NCORES)
    ]

    nc = _get_program()
    res = run_bass_kernel_spmd(nc, in_maps, core_ids=list(range(NCORES)))
    _CACHE["last_results"] = res

    out_density = np.concatenate(
        [res.results[k]["outd"].transpose(1, 0, 2) for k in range(NCORES)],
        axis=0)
    new_cached = np.concatenate(
        [res.results[k]["outc"].transpose(1, 0, 2) for k in range(NCORES)],
        axis=0)
    stat_min = float(
        min(res.results[k]["stats"].min() for k in range(NCORES)))

    if stat_min > MTHR:
        # every voxel has an in-window pair with m > MTHR > -100*ln(0.99),
        # so field > 0.01 >= min(mean, 0.01) everywhere -> mask all-True
        # -> the reference CCL converges to all-True exactly.
        new_field = np.ones((G, G, G), dtype=bool)
    else:
        new_field = _numpy_new_field(density, density_cached)

    valid = new_field if step_i < 500 else old_field
    return (out_density, valid, new_field, new_cached)
